# revision 12
# baseline (speedup 1.0000x reference)
# Trainium2 Bass kernel for nn_CompCSD (segment_reduce):
#   vmf = softmax(vmf_activations, axis=K)
#   content[b,l,h,w]  = sum_{k: label[k]==l} vmf[b,k,h,w]
#   features[b,c,h,w] = sum_k vmf[b,k,h,w] * content[b,label[k],h,w] * kernels[k,c]
#
# Sharding: 8 cores, data-parallel over (batch, H-half): core i -> b=i//2,
# h0=(i%2)*64.  Per core: pixels = 64*128 = 8192, K=256, C=64, L=8.
#
# Device layout per core ("layout B"): K on partitions (2 tiles of 128),
# pixels on the free axis, processed in 16 chunks of 512 pixels.
# Per chunk:
#   e = exp(a)                                  (ACT, one op on [128,2,512])
#   cu9T[pix,j,l] = sum_k e[k,pix] * oh9[k,l]   (PE, 8 small matmuls -> PSUM;
#       col 8 of oh9 is all-ones so cu9T[:,:,8] is the softmax denominator D
#       (col 9 is zero padding: fp32r ISA needs even innermost free extents),
#       cols 0..7 are the per-segment sums, all already transposed to
#       pixel-on-partition layout so the per-pixel normalizers are cheap)
#   invdT = 1/D ; i2T = invdT^2                 (DVE, tiny [128,4] ops)
#   contentT = cu9T[:,:,0:8] * invdT            (DVE, -> SBUF accumulator)
#   cnT      = cu9T[:,:,0:8] * i2T              (DVE)
#   cn       = transpose(cnT) -> [8, 512]       (PE transpose via identity)
#   cg[k,pix] = sum_l sel[l,k] * cn[l,pix]      (PE, gathers cn[label[k]])
#   scaled = e * cg                             (DVE, [128,2,512])
#   fu[c,pix] = sum_k kern[k,c] * scaled[k,pix] (PE -> PSUM, already normalized)
#   feat out via ACT copy PSUM->SBUF + DMA
# content is written once at the end in transposed layout and fixed on host.
#
# Matmul inputs are viewed as float32r (single-pass fp32 on the PE array,
# 1 cycle/row at N>=256 vs 4 for plain fp32).

import numpy as np

B, K, H, W, C = 4, 256, 128, 128, 64
L = 8
NCORES = 8
PIX = H * W // 2        # 8192 pixels per core
NPIX = 512              # pixels per chunk
NCHUNK = PIX // NPIX    # 16
KT = 2                  # K tiles of 128
USE_F32R = True

_prog_cache = {}


def _build_program(use_f32r=USE_F32R, rep=1):
    import concourse.bass as bass
    import concourse.mybir as mybir
    import concourse.tile as tile
    from concourse import bacc
    from concourse.masks import make_identity

    f32 = mybir.dt.float32
    nc = bacc.Bacc("TRN2", target_bir_lowering=False)

    vmf = nc.dram_tensor("vmf", [K, PIX], f32, kind="ExternalInput")
    oh9 = nc.dram_tensor("oh9", [128, KT, L + 2], f32, kind="ExternalInput")
    sel = nc.dram_tensor("sel", [L, KT, 128], f32, kind="ExternalInput")
    kern = nc.dram_tensor("kern", [128, KT, C], f32, kind="ExternalInput")
    feat = nc.dram_tensor("feat", [C, PIX], f32, kind="ExternalOutput")
    contT = nc.dram_tensor("contT", [128, NCHUNK * 4, L], f32, kind="ExternalOutput")

    def r(ap):
        # f32r view: used on matmul inputs AND on the producing instruction's
        # output (walrus checkMatmultFP32r requires producers of f32r matmul
        # inputs to emit float32r, i.e. "rounded").
        return ap.bitcast(mybir.dt.float32r) if use_f32r else ap

    with tile.TileContext(nc) as tc:
        with (
            tc.tile_pool(name="consts", bufs=1) as consts,
            tc.tile_pool(name="io", bufs=3) as io,
            tc.tile_pool(name="work", bufs=2) as work,
            tc.tile_pool(name="accp", bufs=1) as accp,
            tc.tile_pool(name="ps_small", bufs=2, space="PSUM") as ps_small,
            tc.tile_pool(name="ps_big", bufs=1, space="PSUM") as ps_big,
            tc.tile_pool(name="ps_fu", bufs=2, space="PSUM") as ps_fu,
        ):
            sb_oh9 = consts.tile([128, KT, L + 2], f32)
            nc.sync.dma_start(out=r(sb_oh9), in_=r(oh9[:, :, :]))
            sb_sel = consts.tile([L, KT, 128], f32)
            nc.sync.dma_start(out=r(sb_sel), in_=r(sel[:, :, :]))
            sb_kern = consts.tile([128, KT, C], f32)
            nc.sync.dma_start(out=r(sb_kern), in_=r(kern[:, :, :]))
            ident = consts.tile([128, 128], f32)
            make_identity(nc, ident)

            contT_acc = accp.tile([128, NCHUNK * 4, L], f32)

            vmf_r = vmf[:, :].rearrange("(t p) x -> p t x", t=KT)

            for c in [ci for _ in range(rep) for ci in range(NCHUNK)]:
                xs = bass.ds(c * NPIX, NPIX)

                e_in = io.tile([128, KT, NPIX], f32)
                nc.sync.dma_start(out=e_in, in_=vmf_r[:, :, xs])

                e = work.tile([128, KT, NPIX], f32)
                nc.scalar.activation(
                    out=r(e), in_=e_in, func=mybir.ActivationFunctionType.Exp
                )

                cu9T = ps_small.tile([128, 4, L + 2], f32)
                for j in range(4):
                    for t in range(KT):
                        nc.tensor.matmul(
                            cu9T[:, j, :],
                            r(e[:, t, bass.ds(j * 128, 128)]),
                            r(sb_oh9[:, t, :]),
                            start=(t == 0),
                            stop=(t == KT - 1),
                        )

                invdT = work.tile([128, 4], f32)
                nc.vector.reciprocal(out=invdT, in_=cu9T[:, :, L])
                i2T = work.tile([128, 4], f32)
                nc.vector.tensor_mul(i2T, invdT, invdT)

                nc.vector.tensor_mul(
                    contT_acc[:, c * 4 : (c + 1) * 4, :],
                    cu9T[:, :, 0:L],
                    invdT[:, :, None].broadcast_to([128, 4, L]),
                )
                cnT = work.tile([128, 4, L], f32)
                nc.vector.tensor_mul(
                    cnT,
                    cu9T[:, :, 0:L],
                    i2T[:, :, None].broadcast_to([128, 4, L]),
                )

                cn_ps = ps_small.tile([L, 4, 128], f32)
                for j in range(4):
                    nc.tensor.transpose(cn_ps[:, j, :], cnT[:, j, :], ident)
                cn_sb = work.tile([L, 4, 128], f32)
                nc.scalar.copy(out=r(cn_sb), in_=cn_ps)

                cg = ps_big.tile([128, KT, NPIX], f32)
                for t in range(KT):
                    nc.tensor.matmul(
                        cg[:, t, :],
                        r(sb_sel[:, t, :]),
                        r(cn_sb[:, :, :]),
                        start=True,
                        stop=True,
                    )

                scaled = work.tile([128, KT, NPIX], f32)
                nc.vector.tensor_mul(r(scaled), e, cg)

                fu = ps_fu.tile([C, NPIX], f32)
                for t in range(KT):
                    nc.tensor.matmul(
                        fu,
                        r(sb_kern[:, t, :]),
                        r(scaled[:, t, :]),
                        start=(t == 0),
                        stop=(t == KT - 1),
                    )
                fu_sb = io.tile([C, NPIX], f32)
                nc.scalar.copy(out=fu_sb, in_=fu)
                nc.gpsimd.dma_start(out=feat[:, xs], in_=fu_sb)

            nc.gpsimd.dma_start(out=contT[:, :, :], in_=contT_acc)

    nc.finalize()
    return nc


def _get_program(rep=1):
    key = ("prog", USE_F32R, rep)
    if key not in _prog_cache:
        _prog_cache[key] = _build_program(rep=rep)
    return _prog_cache[key]


def _make_consts(kernels, labels):
    oh9 = np.zeros((128, KT, L + 2), np.float32)
    sel = np.zeros((L, KT, 128), np.float32)
    kern = np.zeros((128, KT, C), np.float32)
    ar = np.arange(128)
    for t in range(KT):
        lab_t = labels[t * 128 : (t + 1) * 128]
        oh9[ar, t, lab_t] = 1.0
        oh9[:, t, L] = 1.0
        sel[lab_t, t, ar] = 1.0
        kern[:, t, :] = kernels[t * 128 : (t + 1) * 128, :]
    return oh9, sel, kern


def _run(inputs, trace=False):
    from concourse.bass_utils import run_bass_kernel_spmd

    vmf = np.ascontiguousarray(np.asarray(inputs["vmf_activations"], dtype=np.float32))
    kernels = np.asarray(inputs["kernels"], dtype=np.float32)
    labels = np.asarray(inputs["kernel_labels"]).astype(np.int64)

    oh9, sel, kern = _make_consts(kernels, labels)

    in_maps = []
    for i in range(NCORES):
        b, h0 = i // 2, (i % 2) * 64
        shard = np.ascontiguousarray(vmf[b, :, h0 : h0 + 64, :].reshape(K, PIX))
        in_maps.append({"vmf": shard, "oh9": oh9, "sel": sel, "kern": kern})

    nc = _get_program()
    res = run_bass_kernel_spmd(nc, in_maps, core_ids=list(range(NCORES)), trace=trace)

    content = np.zeros((B, L, H, W), np.float32)
    features = np.zeros((B, C, H, W), np.float32)
    for i, rd in enumerate(res.results):
        b, h0 = i // 2, (i % 2) * 64
        features[b, :, h0 : h0 + 64, :] = rd["feat"].reshape(C, 64, W)
        ct = rd["contT"].reshape(128, NCHUNK, 4, L)
        content[b, :, h0 : h0 + 64, :] = ct.transpose(3, 1, 2, 0).reshape(L, 64, W)
    return (content, features), res


def kernel(**inputs):
    out, _ = _run(inputs, trace=False)
    return out


def _make_in_maps(inputs):
    vmf = np.ascontiguousarray(np.asarray(inputs["vmf_activations"], dtype=np.float32))
    kernels = np.asarray(inputs["kernels"], dtype=np.float32)
    labels = np.asarray(inputs["kernel_labels"]).astype(np.int64)
    oh9, sel, kern = _make_consts(kernels, labels)
    in_maps = []
    for i in range(NCORES):
        b, h0 = i // 2, (i % 2) * 64
        shard = np.ascontiguousarray(vmf[b, :, h0 : h0 + 64, :].reshape(K, PIX))
        in_maps.append({"vmf": shard, "oh9": oh9, "sel": sel, "kern": kern})
    return in_maps


def _make_timing_fn(nc, in_maps):
    """Build a non-donating jitted runner for nc; returns (fn, dev_args)."""
    import jax
    from jax.sharding import Mesh, PartitionSpec
    from jax.experimental.shard_map import shard_map
    import concourse.mybir as mybir
    from concourse import bass2jax

    bass2jax.install_neuronx_cc_hook()

    partition_name = nc.partition_id_tensor.name if nc.partition_id_tensor else None
    in_names, out_names, out_avals, zero_outs = [], [], [], []
    for alloc in nc.m.functions[0].allocations:
        if not isinstance(alloc, mybir.MemoryLocationSet):
            continue
        name = alloc.memorylocations[0].name
        if alloc.kind == "ExternalInput":
            if name != partition_name:
                in_names.append(name)
        elif alloc.kind == "ExternalOutput":
            shape = tuple(alloc.tensor_shape)
            dtype = mybir.dt.np(alloc.dtype)
            out_names.append(name)
            out_avals.append(jax.core.ShapedArray(shape, dtype))
            zero_outs.append(np.zeros(shape, dtype))
    n_params = len(in_names)
    all_in_names = in_names + out_names
    if partition_name is not None:
        all_in_names = all_in_names + [partition_name]

    def _body(*args):
        operands = list(args)
        if partition_name is not None:
            operands.append(bass2jax.partition_id_tensor())
        outs = bass2jax._bass_exec_p.bind(
            *operands,
            out_avals=tuple(out_avals),
            in_names=tuple(all_in_names),
            out_names=tuple(out_names),
            lowering_input_output_aliases=(),
            sim_require_finite=True,
            sim_require_nnan=True,
            nc=nc,
        )
        return tuple(outs)

    devices = jax.devices()[:NCORES]
    mesh = Mesh(np.asarray(devices), ("core",))
    n_outs = len(out_names)
    in_specs = (PartitionSpec("core"),) * (n_params + n_outs)
    out_specs = (PartitionSpec("core"),) * n_outs
    fn = jax.jit(
        shard_map(_body, mesh=mesh, in_specs=in_specs, out_specs=out_specs,
                  check_rep=False),
        keep_unused=True,
    )
    concat_in = [
        np.concatenate([np.asarray(m[nm]) for m in in_maps], axis=0)
        for nm in in_names
    ]
    concat_zero = [
        np.zeros((NCORES * z.shape[0], *z.shape[1:]), z.dtype) for z in zero_outs
    ]
    args = [jax.device_put(a) for a in concat_in + concat_zero]
    return fn, args


def _time_fn(fn, args, iters, warmup=3):
    import jax
    import time as _time

    for _ in range(warmup):
        outs = fn(*args)
    jax.block_until_ready(outs)
    best = float("inf")
    for _ in range(3):
        t0 = _time.perf_counter()
        for _ in range(iters):
            outs = fn(*args)
        jax.block_until_ready(outs)
        best = min(best, (_time.perf_counter() - t0) / iters)
    return best * 1e9


def time_hw(inputs, iters=30, repn=5):
    """Device time per workload pass, measured as the marginal wall-clock cost
    of extra on-device repetitions: (T(repn) - T(1)) / (repn - 1).  All fixed
    per-dispatch overheads (axon round trip, DGE setup, input DMA from host)
    cancel in the difference."""
    in_maps = _make_in_maps(inputs)
    fn1, args1 = _make_timing_fn(_get_program(rep=1), in_maps)
    fnN, argsN = _make_timing_fn(_get_program(rep=repn), in_maps)
    t1 = _time_fn(fn1, args1, iters)
    tN = _time_fn(fnN, argsN, iters)
    print(f"  [time_hw] T(rep=1)={t1:.0f} ns, T(rep={repn})={tN:.0f} ns")
    return (tN - t1) / (repn - 1)
